# revision 1
# baseline (speedup 1.0000x reference)
"""Trainium2 Bass kernel for LogitBiasedSelfAttention1D.

Sharding: 8 cores = (batch b in 0..3) x (query half qh in 0..1).
Each core computes full attention (all 8 heads, all 2048 keys) for the
1024 queries of its batch half. No collectives.

Math decomposition (exactly equivalent to the reference up to fp):
  - conv1d key bias folded into V:  softmax(S + bias) @ V
      = (exp(S) @ (c * V)) / (exp(S) @ c),   c = exp(bias)
  - SCALE folded into w_q on host.
  - b_out + residual x_seq folded into one host-prepared addend.
  - LayerNorm gamma/beta folded into the final transpose drain.
All matmuls in bf16; accumulation and softmax denominator in fp32.

Schedule: software-pipelined.  The attention k-loop is Act-bound (exp);
projection / out-proj matmul "feeder" quanta are interleaved between
attention steps to fill PE slack.  PSUM: SA/SB double-buffered S tiles
(2 banks each), one 3-bank accumulator holding all 16 (head, q-block)
PV blocks of a pair, and one feeder bank.  Pair-boundary normalize is
split DVE/Act and the PE transposes are deferred past the next pair's
first step to avoid head-of-line blocking.
"""

import sys

for _p in ("/opt/trn_rl_repo", "/root/.axon_site/_ro/trn_rl_repo"):
    if _p not in sys.path:
        sys.path.insert(0, _p)

import numpy as np
import ml_dtypes

from concourse import bass, mybir
from concourse.tile import TileContext
from concourse.bass_utils import run_bass_kernel_spmd

B, C, T = 4, 512, 2048
H, D = 8, 64
SCALE = D ** -0.5
EPS = 1e-5
TQ = T // 2            # queries per core
KC = T // 128          # 16 key chunks
PAIRS = H // 2         # 4 head pairs
F32 = mybir.dt.float32
BF16 = mybir.dt.bfloat16
bf16 = ml_dtypes.bfloat16

Exp = mybir.ActivationFunctionType.Exp
Sqrt = mybir.ActivationFunctionType.Sqrt
Square = mybir.ActivationFunctionType.Square
Ident = mybir.ActivationFunctionType.Identity
MULT = mybir.AluOpType.mult
ADD = mybir.AluOpType.add

_CACHE = {}


def _bcol(b):
    """Column offset of 65-wide PV block b (0..15) in the 3-bank OC tile.
    7 + 7 + 2 blocks per bank; no block crosses a 512-col bank boundary."""
    if b < 7:
        return b * 65
    if b < 14:
        return 512 + (b - 7) * 65
    return 1024 + (b - 14) * 65


def _build_nc():
    nc = bass.Bass()
    # packed layouts: one DMA per logical tensor; [128, n*512] with the
    # 128-row blocks of the original (rows, cols) tensor side by side.
    # Token chunks are rotated per core so this core's query half is always
    # chunks j=0,1 (softmax is key-order invariant; the per-key bias c is
    # rotated to match), so Q-projection reads XC directly — no xq input.
    xct = nc.declare_dram_parameter("xct", [128, 4 * T], BF16, False)  # 4 j-chunks
    xseq = nc.declare_dram_parameter("xseq", [TQ, C], F32, False)     # x[b].T + b_out
    wq = nc.declare_dram_parameter("wq", [128, 4 * C], BF16, False)   # (c_in, c_out)*SCALE
    wk = nc.declare_dram_parameter("wk", [128, 4 * C], BF16, False)
    wv = nc.declare_dram_parameter("wv", [128, 4 * C], BF16, False)
    wo = nc.declare_dram_parameter("wo", [C, C], BF16, False)
    cful = nc.declare_dram_parameter("cful", [128, KC], F32, False)   # exp(bias)
    c8 = nc.declare_dram_parameter("c8", [128, KC * H], BF16, False)  # per head
    gmm = nc.declare_dram_parameter("gmm", [128, 4], F32, False)
    bet = nc.declare_dram_parameter("bet", [128, 4], F32, False)
    iden = nc.declare_dram_parameter("iden", [128, 128], BF16, False)
    outp = nc.declare_dram_parameter("out", [C, TQ], F32, True)

    with TileContext(nc) as tc:
        with (
            tc.sbuf_pool(name="cst", bufs=1) as cst,
            tc.sbuf_pool(name="pex", bufs=7) as pex,
            tc.sbuf_pool(name="sml", bufs=2) as sml,
            tc.psum_pool(name="ps", bufs=1) as ps,
        ):
            # ---- critical-path constants, in DMA priority order ----
            # WK4/WQ4/WV4: [128, ci*512+cout]; XC[j]: [128, ci*512+tok]
            # — each a single DMA.
            # wk/wq are packed m-major and split so the first matmuls only
            # wait on the m=0 quarter (128KB) instead of the full 512KB
            ID = cst.tile_from(iden[:, :], name="ID")
            WKa = cst.tile_from(wk[:, 0:512], name="WKa")
            XC = [None] * 4
            XC[0] = cst.tile_from(xct[:, 0:2048], name="XCj0")
            WQa = cst.tile_from(wq[:, 0:512], name="WQa")
            XC[1] = cst.tile_from(xct[:, 2048:4096], name="XCj1")
            WKb = cst.tile_from(wk[:, 512:2048], name="WKb")
            WQb = cst.tile_from(wq[:, 512:2048], name="WQb")
            WV4 = cst.tile_from(wv[:, :], name="WV4")
            WK = (WKa, WKb)
            WQ = (WQa, WQb)
            CF = cst.tile_from(cful[:, :], name="CF")
            C8 = cst.tile_from(c8[:, :], name="C8")
            for j in range(2, 4):
                XC[j] = cst.tile_from(xct[:, j * 2048:(j + 1) * 2048],
                                      name=f"XCj{j}")

            # PE p-state warmup: chain dummy transposes while the first
            # input DMAs stream in, so the prelude matmuls run at full
            # clock instead of the cold 0.65 GHz p-state.
            warm = ps.tile([128, 128], BF16, tag="FA", name="warm")
            for _ in range(48):
                nc.tensor.transpose(warm[:, :], ID[:, :], ID[:, :])

            # ---- persistent SBUF tiles ----
            KT = [cst.tile([128, T], BF16, name=f"KT{m}") for m in range(4)]
            QT = [cst.tile([128, TQ], BF16, name=f"QT{m}") for m in range(4)]
            VB = [cst.tile([128, H * 65], BF16, name=f"VB{k}") for k in range(KC)]
            OT = [cst.tile([128, TQ], BF16, name=f"OTp{p}") for p in range(PAIRS)]
            OACC = [cst.tile([128, C], F32, name=f"OACC{t}") for t in range(8)]

            for k in range(KC):
                nc.gpsimd.tensor_copy(
                    VB[k].rearrange("p (h e) -> p h e", e=65)[:, :, 64:65],
                    C8[:, k * H:(k + 1) * H].rearrange("p (h e) -> p h e", e=1))

            # ---- feeder machinery ----
            # Each feeder quantum carries a key; ensure(key) emits queued
            # quanta (FIFO) until `key` has run.  Correctness never depends
            # on the pump pacing.
            feeders = []
            done = set()

            def pump(n=1):
                for _ in range(n):
                    if feeders:
                        key, fn = feeders.pop(0)
                        fn()
                        done.add(key)

            def ensure(key):
                while key not in done:
                    assert feeders, f"missing feeder quantum {key}"
                    k2, fn = feeders.pop(0)
                    fn()
                    done.add(k2)

            def kq_quantum(dst, Wab, m, j, tag="FA", act_copy=False):
                # dst[:, j*512:(j+1)*512] = W[:, m-block].T @ x-cols-j
                # Wab = (m0-tile, m123-tile), both m-major: [.., m*512+ci*128]
                def emit():
                    W = Wab[0] if m == 0 else Wab[1]
                    c0 = (0 if m == 0 else (m - 1) * 512)
                    fps = ps.tile([128, 512], F32, tag=tag,
                                  name=f"f_{dst.tensor.name}_{j}")
                    for ci in range(4):
                        nc.tensor.matmul(
                            fps[:, :],
                            lhsT=W[:, c0 + ci * 128: c0 + (ci + 1) * 128],
                            rhs=XC[j][:, ci * 512:(ci + 1) * 512],
                            start=(ci == 0), stop=(ci == 3))
                    if act_copy:    # Act is idle at the head — drain there
                        nc.scalar.copy(dst[:, j * 512:(j + 1) * 512], fps[:, :])
                    else:
                        nc.vector.tensor_copy(dst[:, j * 512:(j + 1) * 512],
                                              fps[:, :])
                return emit

            def v_quantum(k, tag="FA"):
                def emit():
                    fps = ps.tile([128, 512], F32, tag=tag, name=f"fv{k}")
                    for ci in range(4):
                        nc.tensor.matmul(
                            fps[:, :],
                            lhsT=XC[k // 4][:, ci * 512 + (k % 4) * 128:
                                            ci * 512 + (k % 4) * 128 + 128],
                            rhs=WV4[:, ci * 512:(ci + 1) * 512],
                            start=(ci == 0), stop=(ci == 3))
                    nc.vector.tensor_scalar(
                        out=VB[k].rearrange("p (h e) -> p h e", e=65)[:, :, 0:64],
                        in0=fps.rearrange("p (h e) -> p h e", e=64),
                        scalar1=CF[:, k:k + 1], scalar2=None, op0=MULT)
                return emit

            # ---- prelude: just enough for pair 0 step 0 (S of chunk 0) ----
            kq_quantum(KT[0], WK, 0, 0, "FA", act_copy=True)()
            kq_quantum(QT[0], WQ, 0, 0, "SA")()
            kq_quantum(QT[0], WQ, 0, 1, "SB", act_copy=True)()
            done.update({("K", 0, 0), ("Q", 0, 0), ("Q", 0, 1)})

            # ---- feeder queue (V_k due at step k+1; KT0 j due at step 4j;
            # KT[m] j>=1 is only needed from step 4j of pair m, so those
            # quanta ride inside pair m instead of loading pair 0) ----
            feeders.append((("V", 0), v_quantum(0)))
            feeders.append((("V", 1), v_quantum(1)))
            feeders.append((("K", 0, 1), kq_quantum(KT[0], WK, 0, 1)))
            feeders.append((("V", 2), v_quantum(2)))
            feeders.append((("V", 3), v_quantum(3)))
            feeders.append((("K", 0, 2), kq_quantum(KT[0], WK, 0, 2)))
            feeders.append((("V", 4), v_quantum(4)))
            feeders.append((("V", 5), v_quantum(5)))
            feeders.append((("K", 0, 3), kq_quantum(KT[0], WK, 0, 3)))
            for k in range(6, KC):
                feeders.append((("V", k), v_quantum(k)))
            for m in range(1, 4):
                feeders.append((("K", m, 0), kq_quantum(KT[m], WK, m, 0)))
                for j in range(2):
                    feeders.append((("Q", m, j),
                                    kq_quantum(QT[m], WQ, m, j)))
                for j in range(1, 4):
                    feeders.append((("K", m, j),
                                    kq_quantum(KT[m], WK, m, j)))

            accs = {}

            def oproj_quantum(p, t):
                def emit():
                    # the last pair's out-proj runs in the tail where the
                    # S banks are free — keep it out of the busy FA slot
                    tag = ("SA", "SB")[t % 2] if p == PAIRS - 1 else "FA"
                    fps = ps.tile([128, 512], F32, tag=tag, name=f"fo{p}_{t}")
                    nc.tensor.matmul(
                        fps[:, :],
                        lhsT=OT[p][:, t * 128:(t + 1) * 128],
                        rhs=WO[p][:, :],
                        start=True, stop=True)
                    in1 = XS[t] if p == 0 else OACC[t]
                    acc = None
                    if p == PAIRS - 1:
                        # free row-sum of the final h -> LN mean
                        acc = sml.tile([128, 1], F32, tag="acc",
                                       name=f"acc{t}", bufs=4)
                        accs[t] = acc
                    nc.vector.scalar_tensor_tensor(
                        out=OACC[t], in0=fps[:, :], scalar=1.0, in1=in1,
                        op0=MULT, op1=ADD, accum_out=acc)
                return emit

            # deferred epilogue inputs — DMAs queue behind the critical ones
            WO = [cst.tile_from(wo[i * 128:(i + 1) * 128, :], name=f"WO{i}")
                  for i in range(4)]
            XS = [cst.tile_from(xseq[t * 128:(t + 1) * 128, :], name=f"XS{t}")
                  for t in range(8)]
            GM = cst.tile_from(gmm[:, :], name="GM")
            BT = cst.tile_from(bet[:, :], name="BT")
            epsT = cst.tile([128, 1], F32, name="epsT")
            nc.vector.memset(epsT[:, :], EPS)

            # ---- attention: pairs outer, key chunks inner ----
            # PV matmuls for chunk k are emitted one step later (at k+1) so
            # a pair-boundary PV stall (waiting on the previous pair's
            # normalize) never queues ahead of the next S/exp on PE.
            def make_tp_quantum(p, ONs, half, chain=None):
                def emit():
                    tp = ps.tile([128, 512], BF16, tag="FA",
                                 name=f"tp{p}_{half}")
                    for j in range(4):
                        nc.tensor.transpose(
                            tp[:, j * 128:(j + 1) * 128],
                            ONs[half * 4 + j][:, :], ID[:, :])
                    if p == PAIRS - 1:  # tail: keep the DVE chain short
                        nc.scalar.copy(
                            OT[p][:, half * 512:(half + 1) * 512], tp[:, :])
                    else:
                        nc.vector.tensor_copy(
                            OT[p][:, half * 512:(half + 1) * 512], tp[:, :])
                    if chain is not None:
                        # last pair: first tail blocks' oproj go FIRST,
                        # the second normalize half + tp1 ride behind them
                        feeders.insert(0, (("O", p, 1), oproj_quantum(p, 1)))
                        feeders.insert(0, (("O", p, 0), oproj_quantum(p, 0)))
                        feeders.append((("NORM2", p), chain[0]))
                        feeders.append((("TP", p, 1), chain[1]))
                        feeders.append((("O", p, 2), oproj_quantum(p, 2)))
                        feeders.append((("O", p, 3), oproj_quantum(p, 3)))
                    else:
                        for t in range(half * 4, half * 4 + 4):
                            feeders.append((("O", p, t), oproj_quantum(p, t)))
                return emit

            def make_boundary(p, oc):
                # normalize pair p: reciprocal of denominators, scale the
                # V blocks.  Emitted inside pair p+1's first step so the
                # Act-side muls queue BEHIND that step's exps.  Split
                # DVE/Act so the next pair's deferred PV (chunk 0)
                # unblocks quickly.
                def emit():
                    rd16 = sml.tile([128, 16], F32, tag="rd", name=f"rd{p}",
                                    bufs=2)
                    for lo, nblk, b0 in ((0, 7, 0), (512, 7, 7), (1024, 2, 14)):
                        nc.vector.reciprocal(
                            rd16[:, b0:b0 + nblk].rearrange(
                                "p (s e) -> p s e", e=1),
                            oc[:, lo:lo + nblk * 65].rearrange(
                                "p (s e) -> p s e", e=65)[:, :, 64:65])
                    ONs = [sml.tile([128, 128], BF16, tag="on",
                                    name=f"on{p}_{s}", bufs=10)
                           for s in range(8)]

                    # One engine writes BOTH halves of each ONs[s]: the
                    # framework orders accesses to a tile across engines,
                    # so mixed writers serialize the whole normalize chain.
                    def muls(srange, on_act=False):
                        def emit2():
                            for s in srange:
                                for hi in range(2):
                                    b = hi * 8 + s
                                    if on_act:
                                        nc.scalar.mul(
                                            ONs[s][:, hi * 64:(hi + 1) * 64],
                                            oc[:, _bcol(b):_bcol(b) + 64],
                                            rd16[:, b:b + 1])
                                    else:
                                        nc.vector.tensor_scalar_mul(
                                            ONs[s][:, hi * 64:(hi + 1) * 64],
                                            oc[:, _bcol(b):_bcol(b) + 64],
                                            rd16[:, b:b + 1])
                        return emit2

                    if p == PAIRS - 1:
                        # normalize only the first half now (DVE, short path
                        # to tp0); the second half runs on the idle Act
                        # engine, paced behind the first tail blocks
                        muls(range(4))()
                        feeders.insert(0, (
                            ("TP", p, 0),
                            make_tp_quantum(p, ONs, 0,
                                            (muls(range(4, 8)),
                                             make_tp_quantum(p, ONs, 1)))))
                    else:
                        muls(range(8))()
                        feeders.insert(0, (("TP", p, 1),
                                           make_tp_quantum(p, ONs, 1)))
                        feeders.insert(0, (("TP", p, 0),
                                           make_tp_quantum(p, ONs, 0)))
                return emit

            pending_boundary = None
            for p in range(PAIRS):
                oc = ps.tile([128, 1536], F32, tag="OC", name=f"oc{p}")

                def pv_emit(k, pts, oc=oc, p=p):
                    for hi in range(2):
                        head = 2 * p + hi
                        for s in range(8):
                            col = _bcol(hi * 8 + s)
                            nc.tensor.matmul(
                                oc[:, col:col + 65],
                                lhsT=pts[hi][:, s * 128:(s + 1) * 128],
                                rhs=VB[k][:, head * 65:(head + 1) * 65],
                                start=(k == 0), stop=(k == KC - 1))

                ensure(("Q", p, 0))
                ensure(("Q", p, 1))
                lag = []   # PV deferred two steps: normalize(p-1) gets two
                           # full steps before the first deferred PV queues
                for k in range(KC):
                    ensure(("K", p, k // 4))
                    pts = []
                    for hi in range(2):
                        rows = slice(hi * 64, (hi + 1) * 64)
                        s_ps = ps.tile([128, 1024], F32, tag=("SA", "SB")[hi],
                                       name=f"s{p}_{k}_{hi}")
                        for n in range(2):
                            nc.tensor.matmul(
                                s_ps[:, n * 512:(n + 1) * 512],
                                lhsT=KT[p][rows, k * 128:(k + 1) * 128],
                                rhs=QT[p][rows, n * 512:(n + 1) * 512],
                                start=True, stop=True)
                        pt = pex.tile([128, 1024], BF16, tag=("pA", "pB")[hi],
                                      name=f"pt{p}_{k}_{hi}")
                        nc.scalar.activation(pt[:, :], s_ps[:, :], Exp)
                        pts.append(pt)
                    if k == 0 and pending_boundary is not None:
                        pending_boundary()
                        pending_boundary = None
                    lag.append((k, pts))
                    # deep lag mid-pair (normalize WAR slack); drain it
                    # progressively near the pair end so the final PV burst
                    # doesn't sit on the normalize critical path
                    depth = 5 if k < KC - 2 else (3 if k == KC - 2 else 1)
                    while len(lag) > depth:
                        kk, pp = lag.pop(0)
                        ensure(("V", kk))
                        pv_emit(kk, pp)
                    if p == 0:
                        if k < KC - 2:
                            pump(2 if k < 8 else 1)
                    elif k < KC - 2:
                        pump(2 if k < 1 else 1)
                for kk, pp in lag:
                    ensure(("V", kk))
                    pv_emit(kk, pp)
                pending_boundary = make_boundary(p, oc)

            pending_boundary()  # last pair: normalize, then tp via feeders
            ensure(("TP", 3, 0))  # first-half transposes; rest paced in tail

            # ---- tail: two-stage software pipeline ----
            # stage A(t): oproj drain + LN stats (DVE-heavy, sqrt on Act)
            # stage B(t): hn (Pool) + transpose (PE) + gamma/beta (Act) + DMA,
            # emitted with one-step skew so no engine queue head-of-line
            # blocks the next t's stage A.
            nmrs, rstds = [], []

            def tail_a(t):
                ensure(("O", 3, t))
                # Sum-of-squares on Act (has tail slack); mean came free
                # from the oproj STT accum_out.  var = (ssq - acc^2/C)/C.
                sq = sml.tile([128, C], F32, tag="sq", name=f"sq{t}", bufs=2)
                ssq = sml.tile([128, 1], F32, tag="ssq", name=f"ssq{t}", bufs=3)
                nc.scalar.activation(sq[:, :], OACC[t][:, :], Square,
                                     accum_out=ssq[:, :])
                dvar = sml.tile([128, 1], F32, tag="dvar", name=f"dv{t}", bufs=3)
                nc.gpsimd.tensor_scalar(out=dvar[:, :], in0=accs[t],
                                        scalar1=accs[t], scalar2=-1.0 / C,
                                        op0=MULT, op1=MULT)
                nc.vector.scalar_tensor_tensor(
                    out=dvar[:, :], in0=dvar[:, :], scalar=1.0,
                    in1=ssq[:, :], op0=MULT, op1=ADD)
                std = sml.tile([128, 1], F32, tag="std", name=f"std{t}", bufs=3)
                nc.scalar.activation(std[:, :], dvar[:, :], Sqrt,
                                     bias=epsT[:, :], scale=1.0 / C)
                rstd = sml.tile([128, 1], F32, tag="rstd", name=f"rstd{t}", bufs=3)
                nc.vector.reciprocal(rstd[:, :], std[:, :])
                nmr = sml.tile([128, 1], F32, tag="nmr", name=f"nmr{t}", bufs=3)
                nc.gpsimd.tensor_scalar(out=nmr[:, :], in0=accs[t],
                                        scalar1=rstd[:, :], scalar2=-1.0 / C,
                                        op0=MULT, op1=MULT)
                rstds.append(rstd)
                nmrs.append(nmr)

            def tail_b(t):
                hn = sml.tile([128, C], BF16, tag="hn", name=f"hn{t}", bufs=3)
                nc.gpsimd.tensor_scalar(out=hn[:, :], in0=OACC[t][:, :],
                                        scalar1=rstds[t], scalar2=nmrs[t],
                                        op0=MULT, op1=ADD)
                # two ftp/outt tiles so the Act- and DVE-side gamma/beta
                # halves never touch the same tile (cross-engine accesses
                # to one tile serialize)
                fa = ps.tile([128, 256], BF16, tag="OC", name=f"ftpa{t}")
                fb = ps.tile([128, 256], BF16, tag="FA", name=f"ftpb{t}")
                for cc in range(4):
                    dst = fa if cc < 2 else fb
                    nc.tensor.transpose(
                        dst[:, (cc % 2) * 128:(cc % 2) * 128 + 128],
                        hn[:, cc * 128:(cc + 1) * 128], ID[:, :])
                oa = sml.tile([128, 256], F32, tag="outa", name=f"outa{t}", bufs=2)
                ob = sml.tile([128, 256], F32, tag="outb", name=f"outb{t}", bufs=2)
                for cc in range(2):
                    nc.scalar.activation(
                        oa[:, cc * 128:(cc + 1) * 128],
                        fa[:, cc * 128:(cc + 1) * 128],
                        Ident, bias=BT[:, cc:cc + 1], scale=GM[:, cc:cc + 1])
                for cc in range(2, 4):
                    nc.vector.tensor_scalar(
                        out=ob[:, (cc - 2) * 128:(cc - 1) * 128],
                        in0=fb[:, (cc - 2) * 128:(cc - 1) * 128],
                        scalar1=GM[:, cc:cc + 1], scalar2=BT[:, cc:cc + 1],
                        op0=MULT, op1=ADD)
                nc.sync.dma_start(
                    out=outp[0:256, t * 128:(t + 1) * 128].rearrange(
                        "(c p) q -> p c q", p=128),
                    in_=oa.rearrange("p (c q) -> p c q", c=2))
                nc.sync.dma_start(
                    out=outp[256:512, t * 128:(t + 1) * 128].rearrange(
                        "(c p) q -> p c q", p=128),
                    in_=ob.rearrange("p (c q) -> p c q", c=2))

            for t in range(8):
                tail_a(t)
                if t >= 1:
                    tail_b(t - 1)
            tail_b(7)

    _split_mm_waits(nc)
    return nc


def _split_mm_waits(nc):
    """Walrus MM structs carry only one sync wait; move extras to a NoOp."""
    f = nc.m.functions[0]
    for bb in f.blocks:
        il = bb.instructions
        out, changed = [], False
        for i in il:
            si = getattr(i, "sync_info", None)
            tn = type(i).__name__
            splittable = tn.startswith("Inst") and tn not in ("InstNoOp", "InstAllEngineBarrier")
            if (splittable and si is not None
                    and si.on_wait is not None and len(si.on_wait) > 1):
                waits = list(si.on_wait)
                for wi, w in enumerate(waits[:-1]):
                    out.append(mybir.InstNoOp(
                        name=f"{i.name}-wsplit{wi}", engine=i.engine,
                        sync_info=mybir.SyncInfo(on_wait=[w], on_update=[])))
                i.sync_info = mybir.SyncInfo(
                    on_wait=[waits[-1]], on_update=list(si.on_update))
                changed = True
            out.append(i)
        if changed:
            bb.instructions = out


def _prep_inputs(x, sqi, w_qkv, w_out, b_out, w_conv, b_conv, ln_gamma, ln_beta):
    x = np.asarray(x, np.float32)
    sqi = np.asarray(sqi, np.float32)
    w_qkv = np.asarray(w_qkv, np.float32)
    w_out = np.asarray(w_out, np.float32)
    b_out = np.asarray(b_out, np.float32)
    w_conv = np.asarray(w_conv, np.float32)
    b_conv = np.asarray(b_conv, np.float32)
    ln_gamma = np.asarray(ln_gamma, np.float32)
    ln_beta = np.asarray(ln_beta, np.float32)

    sp = np.pad(sqi, ((0, 0), (1, 1)))
    bias = (w_conv[0] * sp[:, :-2] + w_conv[1] * sp[:, 1:-1]
            + w_conv[2] * sp[:, 2:] + b_conv)                    # (B, T)
    c = np.exp(bias).astype(np.float32)

    def pack4(a):
        """(512, n) -> [128, 4*n]: 128-row blocks side by side (one DMA)."""
        n = a.shape[1]
        return a.reshape(4, 128, n).transpose(1, 0, 2).reshape(128, 4 * n)

    def pack_mm(a):
        """(512, 512) -> [128, m*512 + ci*128 + c]: m-major so the m=0
        quarter is a contiguous prefix (separately-DMA'd tile)."""
        return a.reshape(4, 128, 4, 128).transpose(1, 2, 0, 3).reshape(128, 2048)

    wqT = pack_mm(w_qkv[:C].T * SCALE).astype(bf16)
    wkT = pack_mm(w_qkv[C:2 * C].T).astype(bf16)
    wvT = pack4(w_qkv[2 * C:].T).astype(bf16)
    woT = w_out.T.astype(bf16)
    gm = ln_gamma.reshape(4, 128).T.copy().astype(np.float32)
    bt = ln_beta.reshape(4, 128).T.copy().astype(np.float32)
    iden = np.eye(128, dtype=bf16)

    in_maps = []
    for core in range(8):
        b, qh = divmod(core, 2)
        qs = slice(qh * TQ, (qh + 1) * TQ)
        # rotate tokens so this core's query half is chunks j=0,1
        xr = np.roll(x[b], -qh * TQ, axis=1)
        cb = np.roll(c[b], -qh * TQ)
        cfl = cb.reshape(KC, 128).T.copy().astype(np.float32)
        c8m = np.repeat(cb.reshape(KC, 128).T, H, axis=1).copy().astype(bf16)
        xp = np.concatenate(
            [pack4(xr[:, j * 512:(j + 1) * 512]) for j in range(4)], axis=1)
        in_maps.append({
            "xct": xp.astype(bf16),
            "xseq": (x[b].T[qs] + b_out).copy().astype(np.float32),
            "wq": wqT, "wk": wkT, "wv": wvT, "wo": woT,
            "cful": cfl, "c8": c8m, "gmm": gm, "bet": bt, "iden": iden,
        })
    return in_maps


def kernel(x, sqi, w_qkv, w_out, b_out, w_conv, b_conv, ln_gamma, ln_beta,
           _trace=False):
    if "nc" not in _CACHE:
        _CACHE["nc"] = _build_nc()
    nc = _CACHE["nc"]
    in_maps = _prep_inputs(x, sqi, w_qkv, w_out, b_out, w_conv, b_conv,
                           ln_gamma, ln_beta)
    res = run_bass_kernel_spmd(nc, in_maps, core_ids=list(range(8)), trace=_trace)
    _CACHE["last_result"] = res
    out = np.empty((B, C, T), np.float32)
    for core in range(8):
        b, qh = divmod(core, 2)
        out[b][:, qh * TQ:(qh + 1) * TQ] = res.results[core]["out"]
    return out

